# revision 6
# baseline (speedup 1.0000x reference)
"""Trainium2 Bass kernel for nn_CA_SAGE (GNN message passing + MLP head).

Strategy (8 NeuronCores, SPMD single program, per-core data):
  - Nodes sharded across cores (degree-sorted round-robin) for the
    151K-edge weighted scatter-mean: per-core ELL gather via dma_gather
    (256B rows of x), sigmoid(edge weights) + multiply + segment reduce
    on-device.
  - LayerNorm stats via small AllReduce; conv stack via block-diagonal
    TensorE matmuls; flattened features AllGathered (bf16) so fc1/fc2
    run output-sharded (weights sharded 8-way, bf16); final logits
    AllReduced; softmax on-device.
All floating-point model math runs on device. Host does index
preprocessing, sharding/relayout, and weight down-cast to bf16 only.
"""
import sys, os

sys.path.insert(0, "/opt/trn_rl_repo")

import numpy as np

import concourse.bass as bass
import concourse.bacc as bacc
import concourse.tile as tile
import concourse.mybir as mybir
from concourse import bass_utils, masks

N, B, E, MID = 2207, 64, 151215, 8
NCORES = 8
NPC = 276                      # nodes per core (last core has 275)
TILES = (128, 128, 20)         # node tiles per core (partition counts)
NTILES = 3
F1, F2 = 2048, 1024
F1S, F2S = F1 // NCORES, F2 // NCORES   # 256, 128
KT = 72                        # fc1 k-tiles: 8 cores x (2 full + 1 tail slot)
EPS = 1e-5
LN_CNT = float(N * MID)

f32 = mybir.dt.float32
bf16 = mybir.dt.bfloat16
i16 = mybir.dt.int16
AF = mybir.ActivationFunctionType
OP = mybir.AluOpType

_cache = {}


# --------------------------------------------------------------------------
# host-side preprocessing: index work, sharding, relayout only
# --------------------------------------------------------------------------
def _prep(inputs):
    x = np.ascontiguousarray(np.asarray(inputs["x"], dtype=np.float32)[:, :, 0])
    ei = np.asarray(inputs["edge_index"]).astype(np.int64)
    src, dst = ei[0], ei[1]
    ewr = np.asarray(inputs["edge_weight_raw"], dtype=np.float32)

    deg = np.bincount(dst, minlength=N)
    order = np.argsort(-deg, kind="stable")          # nodes by degree desc
    csr_order = np.argsort(dst, kind="stable")       # edges sorted by dst
    src_s = src[csr_order]
    ewr_s = ewr[csr_order]
    starts = np.zeros(N + 1, dtype=np.int64)
    np.cumsum(deg, out=starts[1:])

    core_nodes = [order[c::NCORES] for c in range(NCORES)]  # len 276 / 275

    # per-tile K (max degree in tile, same across cores -> same program)
    bounds = [(0, 128), (128, 256), (256, 276)]
    K = []
    for lo, hi in bounds:
        k = 1
        for c in range(NCORES):
            sl = core_nodes[c][lo:hi]
            if len(sl):
                k = max(k, int(deg[sl].max()))
        K.append(k)
    K = tuple(K)

    per_core = []
    W1P = None
    for c in range(NCORES):
        nodes = core_nodes[c]
        nv = len(nodes)
        gidx = []
        gw = np.full((128, sum(K)), -1e30, dtype=np.float32)
        denom = np.ones((128, NTILES), dtype=np.float32)
        onesn = np.zeros((128, NTILES), dtype=np.float32)
        x_loc = np.zeros((128, NTILES, B), dtype=np.float32)
        lng = np.ones((128, NTILES, MID), dtype=np.float32)
        lnb = np.zeros((128, NTILES, MID), dtype=np.float32)
        koff = 0
        for t, (lo, hi) in enumerate(bounds):
            kt = K[t]
            lin = np.zeros(kt * 128, dtype=np.int16)
            for p in range(min(hi, nv) - lo):
                n = nodes[lo + p]
                d = deg[n]
                s0 = starts[n]
                lin[p::128][:d] = src_s[s0:s0 + d]
                gw[p, koff:koff + d] = ewr_s[s0:s0 + d]
                denom[p, t] = max(d, 1)
                onesn[p, t] = 1.0
                x_loc[p, t] = x[n]
                lng[p, t] = inputs["ln_g"][n]
                lnb[p, t] = inputs["ln_b"][n]
            w16 = np.zeros((16, kt * 8), dtype=np.int16)
            for i in range(kt * 128):
                w16[i % 16, i // 16] = lin[i]
            gidx.append(np.tile(w16, (8, 1)))
            koff += kt
        per_core.append(dict(gidx=gidx, gw=gw, denom=denom, onesn=onesn,
                             x_loc=x_loc.reshape(128, -1),
                             lng=lng.reshape(128, -1), lnb=lnb.reshape(128, -1)))

    # fc1 weight row packing to match the AllGather layout:
    #   row r = core*1152 + (c4*2 + t)*128 + p      (t in {0,1})
    #   row r = core*1152 + 1024 + c4*32 + p'       (tile-2 tail, p' < 32)
    fc1_w = np.asarray(inputs["fc1_w"], dtype=np.float32)
    W1P = np.zeros((KT * 128, F1), dtype=np.float32)
    for c in range(NCORES):
        nodes = core_nodes[c]
        nv = len(nodes)
        for c4 in range(4):
            for t in range(2):
                rows = c * 1152 + (c4 * 2 + t) * 128
                nd = nodes[t * 128:(t + 1) * 128]
                W1P[rows:rows + 128] = fc1_w[c4 * N + nd]
            nt = nodes[256:]
            rows = c * 1152 + 1024 + c4 * 32
            W1P[rows:rows + len(nt)] = fc1_w[c4 * N + nt]

    import ml_dtypes
    bft = ml_dtypes.bfloat16
    fc2_w = np.asarray(inputs["fc2_w"], dtype=np.float32)
    fco_w = np.asarray(inputs["fco_w"], dtype=np.float32)

    # block-diagonal conv weights (pure relayout with zeros)
    c1 = np.asarray(inputs["c1_w"], dtype=np.float32)   # [12, 8]
    c2 = np.asarray(inputs["c2_w"], dtype=np.float32)   # [4, 12]
    c1bd = np.zeros((128, 96), dtype=np.float32)
    for b in range(8):
        c1bd[b * 8:(b + 1) * 8, b * 12:(b + 1) * 12] = c1.T
    c1bd[64:128] = c1bd[0:64]  # duplicate: lhsT base must match rhs base
    c2bd = np.zeros((96, 32), dtype=np.float32)
    for b in range(8):
        for o2 in range(4):
            c2bd[b * 12:(b + 1) * 12, o2 * 8 + b] = c2[o2]

    row = lambda k: np.asarray(inputs[k], dtype=np.float32).reshape(1, -1)
    shared = dict(
        x_rows=x, c1bd=c1bd, c2bd=c2bd,
        alphas=row("alphas"), selfw=row("sage_self_w"), sagew=row("sage_w"),
        sageb=row("sage_b"), c1b=row("c1_b"), bn1g=row("bn1_g"),
        bn1b=row("bn1_b"), bn1rm=row("bn1_rm"), bn1rv=row("bn1_rv"),
        c2b=row("c2_b"), bn2g=row("bn2_g"), bn2b=row("bn2_b"),
        bn2rm=row("bn2_rm"), bn2rv=row("bn2_rv"), fcob=row("fco_b"),
    )

    in_maps = []
    for c in range(NCORES):
        m = dict(shared)
        m.update(per_core[c])
        m["gidx0"], m["gidx1"], m["gidx2"] = m.pop("gidx")
        m["w1"] = W1P[:, c * F1S:(c + 1) * F1S].astype(bft)
        m["w2"] = fc2_w[:, c * F2S:(c + 1) * F2S].astype(bft)
        m["fcow"] = fco_w[c * F2S:(c + 1) * F2S]
        m["fc1b"] = row("fc1_b")[:, c * F1S:(c + 1) * F1S]
        m["bnf1g"] = row("bnf1_g")[:, c * F1S:(c + 1) * F1S]
        m["bnf1b"] = row("bnf1_b")[:, c * F1S:(c + 1) * F1S]
        m["bnf1rm"] = row("bnf1_rm")[:, c * F1S:(c + 1) * F1S]
        m["bnf1rv"] = row("bnf1_rv")[:, c * F1S:(c + 1) * F1S]
        m["fc2b"] = row("fc2_b")[:, c * F2S:(c + 1) * F2S]
        m["bnf2g"] = row("bnf2_g")[:, c * F2S:(c + 1) * F2S]
        m["bnf2b"] = row("bnf2_b")[:, c * F2S:(c + 1) * F2S]
        m["bnf2rm"] = row("bnf2_rm")[:, c * F2S:(c + 1) * F2S]
        m["bnf2rv"] = row("bnf2_rv")[:, c * F2S:(c + 1) * F2S]
        in_maps.append(m)
    return K, in_maps


# --------------------------------------------------------------------------
# device program
# --------------------------------------------------------------------------
def _fold_bn(nc, pool, name, g, b, rm, rv, eps_ap, extra_b=None):
    """returns (s, t) rows [1, D]: y = z*s + t  (+extra bias folded in)."""
    D = g.shape[1]
    s = pool.tile([1, D], f32, tag=f"fold_{name}_s")
    t = pool.tile([1, D], f32, tag=f"fold_{name}_t")
    tmp = pool.tile([1, D], f32, tag=f"fold_{name}_m")
    nc.scalar.activation(tmp[:], rv, AF.Sqrt, bias=eps_ap)
    nc.vector.reciprocal(tmp[:], tmp[:])
    nc.vector.tensor_tensor(s[:], g, tmp[:], OP.mult)
    if extra_b is None:
        nc.vector.tensor_tensor(tmp[:], rm, s[:], OP.mult)
        nc.vector.tensor_tensor(t[:], b, tmp[:], OP.subtract)
    else:
        nc.vector.tensor_tensor(tmp[:], extra_b, rm, OP.subtract)
        nc.vector.tensor_tensor(tmp[:], tmp[:], s[:], OP.mult)
        nc.vector.tensor_tensor(t[:], b, tmp[:], OP.add)
    return s, t


def _build(K):
    nc = bacc.Bacc("TRN2", target_bir_lowering=False, debug=False,
                   enable_asserts=True, num_devices=NCORES)
    KSUM = sum(K)

    din = {}
    def dt_(name, shape, dtype=f32):
        din[name] = nc.dram_tensor(name, list(shape), dtype, kind="ExternalInput")
        return din[name]

    for t in range(NTILES):
        dt_(f"gidx{t}", (128, K[t] * 8), i16)
    dt_("gw", (128, KSUM)); dt_("denom", (128, NTILES)); dt_("onesn", (128, NTILES))
    dt_("x_loc", (128, NTILES * B)); dt_("lng", (128, NTILES * MID))
    dt_("lnb", (128, NTILES * MID)); dt_("x_rows", (N, B))
    dt_("c1bd", (128, 96)); dt_("c2bd", (96, 32))
    dt_("w1", (KT * 128, F1S), bf16); dt_("w2", (F1, F2S), bf16)
    dt_("fcow", (F2S, 2))
    for nm in ("alphas", "selfw", "sagew", "sageb", "c1b", "bn1g", "bn1b",
               "bn1rm", "bn1rv", "c2b", "bn2g", "bn2b", "bn2rm", "bn2rv",
               "fcob"):
        sz = {"alphas": 2, "selfw": 8, "sagew": 8, "sageb": 8}.get(nm, None)
        if sz is None:
            sz = 12 if "1" in nm and "f" not in nm else 4
            if nm == "fcob":
                sz = 2
        dt_(nm, (1, sz))
    for nm in ("fc1b", "bnf1g", "bnf1b", "bnf1rm", "bnf1rv"):
        dt_(nm, (1, F1S))
    for nm in ("fc2b", "bnf2g", "bnf2b", "bnf2rm", "bnf2rv"):
        dt_(nm, (1, F2S))
    out = nc.dram_tensor("out", [B, 2], f32, kind="ExternalOutput")

    with tile.TileContext(nc) as tc:
        with tc.tile_pool(name="const", bufs=1) as cp, \
             tc.tile_pool(name="work", bufs=2) as wp, \
             tc.tile_pool(name="hbuf", bufs=3) as hp, \
             tc.tile_pool(name="psA", bufs=2, space="PSUM") as psA, \
             tc.tile_pool(name="psAcc", bufs=1, space="PSUM") as psAcc, \
             tc.tile_pool(name="dram", bufs=1, space="DRAM") as dp:

            ident = cp.tile([128, 128], f32, tag="ident")
            masks.make_identity(nc, ident[:])
            ones_row = cp.tile([1, 128], f32, tag="ones_row")
            nc.vector.memset(ones_row[:], 1.0)
            eps_c = cp.tile([1, 1], f32, tag="eps_c")
            nc.vector.memset(eps_c[:], EPS)

            # ---------------- load inputs to SBUF ----------------
            sb = {}
            def ld(name, shape, dtype=f32, engine=None):
                t_ = cp.tile(list(shape), dtype, tag="sb_" + name)
                (engine or nc.sync).dma_start(t_[:], din[name][:])
                sb[name] = t_
                return t_

            for t in range(NTILES):
                ld(f"gidx{t}", (128, K[t] * 8), i16)
            ld("gw", (128, KSUM)); ld("denom", (128, NTILES))
            ld("onesn", (128, NTILES)); ld("x_loc", (128, NTILES, B))
            ld("lng", (128, NTILES, MID)); ld("lnb", (128, NTILES, MID))
            ld("c1bd", (128, 96)); ld("c2bd", (96, 32)); ld("fcow", (F2S, 2))
            for nm in ("alphas", "selfw", "sagew", "sageb", "c1b", "bn1g",
                       "bn1b", "bn1rm", "bn1rv", "c2b", "bn2g", "bn2b",
                       "bn2rm", "bn2rv", "fcob"):
                ld(nm, (1, din[nm].shape[1]))
            for nm in ("fc1b", "bnf1g", "bnf1b", "bnf1rm", "bnf1rv"):
                ld(nm, (1, F1S))
            for nm in ("fc2b", "bnf2g", "bnf2b", "bnf2rm", "bnf2rv"):
                ld(nm, (1, F2S))
            w1sb = cp.tile([128, KT, F1S], bf16, tag="w1sb")
            nc.sync.dma_start(w1sb[:], din["w1"][:].rearrange("(s p) o -> p s o", p=128))
            w2sb = cp.tile([128, 16, F2S], bf16, tag="w2sb")
            nc.sync.dma_start(w2sb[:], din["w2"][:].rearrange("(s p) o -> p s o", p=128))

            # ---------------- preamble: folded constants ----------------
            def to_col(row_ap, m, tag):
                """[1, m] row -> [m, 1] column via K=1 matmul."""
                ps = psA.tile([128, 128], f32, tag="pt")
                nc.tensor.matmul(ps[0:m, 0:1], row_ap, ones_row[0:1, 0:1],
                                 start=True, stop=True)
                col = cp.tile([m, 1], f32, tag="col_" + tag)
                nc.vector.tensor_copy(col[:], ps[0:m, 0:1])
                return col

            a2 = cp.tile([1, 2], f32, tag="a2")
            nc.vector.tensor_scalar_mul(a2[:], sb["alphas"][:], 0.5)
            uvc = cp.tile([1, 24], f32, tag="uvc")
            nc.vector.tensor_scalar_mul(uvc[:, 0:8], sb["selfw"][:], a2[:, 0:1])
            nc.vector.tensor_scalar_mul(uvc[:, 8:16], sb["sagew"][:], a2[:, 1:2])
            nc.vector.tensor_scalar_mul(uvc[:, 16:24], sb["sageb"][:], a2[:, 1:2])
            ps_uvc = psA.tile([128, 128], f32, tag="pt")
            nc.tensor.matmul(ps_uvc[:, 0:24], ones_row[0:1, :], uvc[:], start=True, stop=True)
            uvcr = cp.tile([128, 24], f32, tag="uvcr")
            nc.vector.tensor_copy(uvcr[:], ps_uvc[:, 0:24])

            s1r, t1r = _fold_bn(nc, cp, "bn1", sb["bn1g"][:], sb["bn1b"][:],
                                sb["bn1rm"][:], sb["bn1rv"][:], eps_c[:])
            s2r, t2r = _fold_bn(nc, cp, "bn2", sb["bn2g"][:], sb["bn2b"][:],
                                sb["bn2rm"][:], sb["bn2rv"][:], eps_c[:])
            sf1, tf1 = _fold_bn(nc, cp, "bnf1", sb["bnf1g"][:], sb["bnf1b"][:],
                                sb["bnf1rm"][:], sb["bnf1rv"][:], eps_c[:],
                                extra_b=sb["fc1b"][:])
            sf2, tf2 = _fold_bn(nc, cp, "bnf2", sb["bnf2g"][:], sb["bnf2b"][:],
                                sb["bnf2rm"][:], sb["bnf2rv"][:], eps_c[:],
                                extra_b=sb["fc2b"][:])

            def tile8(row_ap, d, tag):
                # r[b*d + o] = row[o]   (b-outer)
                r = cp.tile([1, 8 * d], f32, tag=tag)
                nc.vector.tensor_copy(
                    r[:].rearrange("o (r d) -> o r d", r=8),
                    row_ap.unsqueeze(1).broadcast_to([1, 8, d]))
                return r

            def rep8(row_ap, d, tag):
                # r[o*8 + b] = row[o]   (o-outer)
                r = cp.tile([1, 8 * d], f32, tag=tag)
                nc.vector.tensor_copy(
                    r[:].rearrange("o (d r) -> o d r", r=8),
                    row_ap.unsqueeze(2).broadcast_to([1, d, 8]))
                return r

            c1bcol = to_col(tile8(sb["c1b"][:], 12, "c1brow")[:], 96, "c1b")
            s1col = to_col(tile8(s1r[:], 12, "s1row")[:], 96, "s1")
            t1col = to_col(tile8(t1r[:], 12, "t1row")[:], 96, "t1")
            c2bcol = to_col(rep8(sb["c2b"][:], 4, "c2brow")[:], 32, "c2b")
            s2col = to_col(rep8(s2r[:], 4, "s2row")[:], 32, "s2")
            t2col = to_col(rep8(t2r[:], 4, "t2row")[:], 32, "t2")
            sf1c = [to_col(sf1[:, 128 * j:128 * (j + 1)], 128, f"sf1_{j}")
                    for j in range(2)]
            tf1c = [to_col(tf1[:, 128 * j:128 * (j + 1)], 128, f"tf1_{j}")
                    for j in range(2)]
            sf2c = to_col(sf2[:], 128, "sf2")
            tf2c = to_col(tf2[:], 128, "tf2")
            fbcol = to_col(sb["fcob"][:], 2, "fcob")

            # ---------------- phase 1: SAGE per node-tile ----------------
            h_tiles = []
            ps_stats = psAcc.tile([1, 128], f32, tag="acc")
            koff = 0
            for t in range(NTILES):
                kt, valid = K[t], TILES[t]
                G = wp.tile([128, kt, B], f32, tag="G")
                nc.gpsimd.dma_gather(
                    out_ap=G[:], in_ap=din["x_rows"][:], idxs_ap=sb[f"gidx{t}"][:],
                    num_idxs=128 * kt, num_idxs_reg=128 * kt, elem_size=B,
                    single_packet=False)
                W = wp.tile([128, kt], f32, tag="W")
                nc.scalar.activation(W[:], sb["gw"][:, koff:koff + kt], AF.Sigmoid)
                nc.vector.tensor_tensor(
                    G[:], G[:], W[:].unsqueeze(2).broadcast_to([128, kt, B]),
                    OP.mult)
                agg = wp.tile([128, B], f32, tag="agg")
                nc.vector.tensor_reduce(
                    agg[0:valid, :],
                    G[0:valid].rearrange("p k b -> p b k"),
                    mybir.AxisListType.X, OP.add)
                dinv = wp.tile([128, 1], f32, tag="dinv")
                nc.vector.reciprocal(dinv[0:valid], sb["denom"][0:valid, t:t + 1])
                nc.vector.tensor_scalar_mul(agg[0:valid], agg[0:valid], dinv[0:valid])

                h = hp.tile([128, B, MID], f32, tag="h")
                h_tiles.append(h)
                va = slice(0, valid)
                xb = sb["x_loc"][va, t, :].unsqueeze(2).broadcast_to([valid, B, MID])
                ub = uvcr[va, 0:8].unsqueeze(1).broadcast_to([valid, B, MID])
                nc.vector.tensor_tensor(h[va], xb, ub, OP.mult)
                tmp = wp.tile([128, B, MID], f32, tag="htmp")
                ab = agg[va].unsqueeze(2).broadcast_to([valid, B, MID])
                vb = uvcr[va, 8:16].unsqueeze(1).broadcast_to([valid, B, MID])
                nc.vector.tensor_tensor(tmp[va], ab, vb, OP.mult)
                nc.vector.tensor_tensor(h[va], h[va], tmp[va], OP.add)
                cb = uvcr[va, 16:24].unsqueeze(1).broadcast_to([valid, B, MID])
                nc.vector.tensor_tensor(h[va], h[va], cb, OP.add)
                nc.vector.tensor_scalar_max(h[va], h[va], 0.0)

                # LN stats partials (masked by onesn to kill pad nodes)
                st = wp.tile([128, 128], f32, tag="st")
                nc.vector.tensor_reduce(st[va, 0:64], h[va],
                                        mybir.AxisListType.X, OP.add)
                hsq = wp.tile([128, B, MID], f32, tag="hsq")
                nc.scalar.activation(hsq[va], h[va], AF.Square)
                nc.vector.tensor_reduce(st[va, 64:128], hsq[va],
                                        mybir.AxisListType.X, OP.add)
                nc.tensor.matmul(ps_stats[:], sb["onesn"][va, t:t + 1], st[va, :],
                                 start=(t == 0), stop=(t == NTILES - 1))
                koff += kt

            # ---------------- LN stats AllReduce ----------------
            stt = cp.tile([1, 128], f32, tag="stt")
            nc.vector.tensor_copy(stt[:], ps_stats[:])
            st_in = dp.tile([1, 128], f32, tag="st_in")
            st_out = dp.tile([1, 128], f32, tag="st_out")
            nc.sync.dma_start(st_in[:], stt[:])
            nc.gpsimd.collective_compute(
                "AllReduce", OP.add, replica_groups=[list(range(NCORES))],
                ins=[st_in[:].opt()], outs=[st_out[:].opt()])
            gst = cp.tile([1, 128], f32, tag="gst")
            nc.sync.dma_start(gst[:], st_out[:])

            murs = cp.tile([1, 128], f32, tag="murs")
            nc.vector.tensor_scalar_mul(murs[:, 0:64], gst[:, 0:64], 1.0 / LN_CNT)
            e2 = cp.tile([1, 64], f32, tag="e2")
            nc.vector.tensor_scalar_mul(e2[:], gst[:, 64:128], 1.0 / LN_CNT)
            var = cp.tile([1, 64], f32, tag="var")
            nc.vector.tensor_tensor(var[:], murs[:, 0:64], murs[:, 0:64], OP.mult)
            nc.vector.tensor_tensor(var[:], e2[:], var[:], OP.subtract)
            nc.scalar.activation(var[:], var[:], AF.Sqrt, bias=eps_c[:])
            nc.vector.reciprocal(murs[:, 64:128], var[:])
            ps_m = psA.tile([128, 128], f32, tag="pt")
            nc.tensor.matmul(ps_m[:], ones_row[0:1, :], murs[:], start=True, stop=True)
            mrep = cp.tile([128, 128], f32, tag="mrep")
            nc.vector.tensor_copy(mrep[:], ps_m[:])

            # ---------------- normalize + convs + transpose ----------------
            hloc = cp.tile([128, NTILES, 4, B], bf16, tag="hloc")
            nc.gpsimd.memset(hloc[:], 0.0)
            for t in range(NTILES):
                kt, valid = K[t], TILES[t]
                va = slice(0, valid)
                h = h_tiles[t]
                mub = mrep[va, 0:64].unsqueeze(2).broadcast_to([valid, B, MID])
                rsb = mrep[va, 64:128].unsqueeze(2).broadcast_to([valid, B, MID])
                gb = sb["lng"][va, t, :].unsqueeze(1).broadcast_to([valid, B, MID])
                bb = sb["lnb"][va, t, :].unsqueeze(1).broadcast_to([valid, B, MID])
                nc.vector.tensor_tensor(h[va], h[va], mub, OP.subtract)
                nc.vector.tensor_tensor(h[va], h[va], rsb, OP.mult)
                nc.vector.tensor_tensor(h[va], h[va], gb, OP.mult)
                nc.vector.tensor_tensor(h[va], h[va], bb, OP.add)

                hT = wp.tile([128, 4, 128], f32, tag="hT")
                for j in range(4):
                    ps_t = psA.tile([128, 128], f32, tag="pt")
                    nc.tensor.transpose(
                        ps_t[:, 0:valid],
                        h[va].rearrange("p b m -> p (b m)")[:, 128 * j:128 * (j + 1)],
                        ident[0:valid, 0:valid])
                    nc.vector.tensor_copy(hT[:, j, 0:valid], ps_t[:, 0:valid])

                for sbi in range(8):
                    j, half = sbi // 2, (sbi % 2) * 64
                    ps1 = psA.tile([96, 128], f32, tag="ptc1")
                    nc.tensor.matmul(ps1[:, 0:valid],
                                     sb["c1bd"][half:half + 64, :],
                                     hT[half:half + 64, j, 0:valid],
                                     start=True, stop=True)
                    nc.scalar.activation(ps1[:, 0:valid], ps1[:, 0:valid],
                                         AF.Relu, bias=c1bcol[:])
                    z1 = wp.tile([96, 128], f32, tag="z1")
                    nc.vector.tensor_scalar(z1[:, 0:valid], ps1[:, 0:valid],
                                            s1col[:], t1col[:], OP.mult, OP.add)
                    ps2 = psA.tile([32, 128], f32, tag="ptc2")
                    nc.tensor.matmul(ps2[:, 0:valid], sb["c2bd"][:], z1[:, 0:valid],
                                     start=True, stop=True)
                    nc.scalar.activation(ps2[:, 0:valid], ps2[:, 0:valid],
                                         AF.Relu, bias=c2bcol[:])
                    h2t = wp.tile([32, 128], f32, tag="h2t")
                    nc.vector.tensor_scalar(h2t[:, 0:valid], ps2[:, 0:valid],
                                            s2col[:], t2col[:], OP.mult, OP.add)
                    ps3 = psA.tile([128, 128], f32, tag="pt")
                    nc.tensor.transpose(ps3[0:valid, 0:32], h2t[:, 0:valid],
                                        ident[0:32, 0:32])
                    nc.vector.tensor_copy(
                        hloc[va, t, :, sbi * 8:(sbi + 1) * 8],
                        ps3[0:valid, 0:32].rearrange("p (c b) -> p c b", c=4))

            # ---------------- h_flat AllGather ----------------
            h_in = dp.tile([1152, B], bf16, tag="h_in")
            h_out = dp.tile([NCORES * 1152, B], bf16, tag="h_out")
            for t in range(2):
                nc.sync.dma_start(
                    h_in[0:1024, :].rearrange("(c t p) b -> p c t b", c=4, t=2)[:, :, t, :],
                    hloc[:, t, :, :])
            nc.sync.dma_start(
                h_in[1024:1152, :].rearrange("(c pp) b -> pp c b", c=4),
                hloc[0:32, 2, :, :].rearrange("p c b -> p c b"))
            nc.gpsimd.collective_compute(
                "AllGather", OP.bypass, replica_groups=[list(range(NCORES))],
                ins=[h_in[:].opt()], outs=[h_out[:].opt()])
            hfT = cp.tile([128, KT, B], bf16, tag="hfT")
            nc.sync.dma_start(hfT[:], h_out[:].rearrange("(s p) b -> p s b", p=128))

            # ---------------- fc1 (out-sharded 256) ----------------
            ps_h1 = psAcc.tile([64, F1S], f32, tag="acc")
            for s in range(KT):
                nc.tensor.matmul(ps_h1[:], hfT[:, s, :], w1sb[:, s, :],
                                 start=(s == 0), stop=(s == KT - 1))
            z1f = cp.tile([64, F1S], f32, tag="z1f")
            nc.vector.tensor_copy(z1f[:], ps_h1[:])
            h1loc = cp.tile([128, 2, B], bf16, tag="h1loc")
            for j in range(2):
                ps_t = psA.tile([128, 128], f32, tag="pt")
                nc.tensor.transpose(ps_t[:, 0:64], z1f[:, 128 * j:128 * (j + 1)],
                                    ident[0:64, 0:64])
                nc.scalar.activation(h1loc[:, j, :], ps_t[:, 0:64], AF.Relu,
                                     bias=tf1c[j][:], scale=sf1c[j][:])

            # ---------------- h1 AllGather + fc2 ----------------
            h1_in = dp.tile([F1S, B], bf16, tag="h1_in")
            h1_out = dp.tile([F1, B], bf16, tag="h1_out")
            nc.sync.dma_start(h1_in[:].rearrange("(k p) b -> p k b", p=128), h1loc[:])
            nc.gpsimd.collective_compute(
                "AllGather", OP.bypass, replica_groups=[list(range(NCORES))],
                ins=[h1_in[:].opt()], outs=[h1_out[:].opt()])
            h1T = cp.tile([128, 16, B], bf16, tag="h1T")
            nc.sync.dma_start(h1T[:], h1_out[:].rearrange("(s p) b -> p s b", p=128))

            ps_h2 = psAcc.tile([128, B], f32, tag="acc")
            for s in range(16):
                nc.tensor.matmul(ps_h2[:], w2sb[:, s, :], h1T[:, s, :],
                                 start=(s == 0), stop=(s == 15))
            h2T = cp.tile([128, B], f32, tag="h2T")
            nc.scalar.activation(h2T[:], ps_h2[:], AF.Relu,
                                 bias=tf2c[:], scale=sf2c[:])

            # ---------------- fco partial + AllReduce + softmax ----------------
            ps_lg = psA.tile([128, 128], f32, tag="pt")
            nc.tensor.matmul(ps_lg[0:2, 0:B], sb["fcow"][:], h2T[:],
                             start=True, stop=True)
            lgp = cp.tile([2, B], f32, tag="lgp")
            nc.vector.tensor_copy(lgp[:], ps_lg[0:2, 0:B])
            lg_in = dp.tile([2, B], f32, tag="lg_in")
            lg_out = dp.tile([2, B], f32, tag="lg_out")
            nc.sync.dma_start(lg_in[:], lgp[:])
            nc.gpsimd.collective_compute(
                "AllReduce", OP.add, replica_groups=[list(range(NCORES))],
                ins=[lg_in[:].opt()], outs=[lg_out[:].opt()])
            lgT = cp.tile([2, B], f32, tag="lgT")
            nc.sync.dma_start(lgT[:], lg_out[:])
            nc.vector.tensor_scalar_add(lgT[:], lgT[:], fbcol[:])
            ps_sm = psA.tile([128, 128], f32, tag="pt")
            nc.tensor.transpose(ps_sm[0:B, 0:2], lgT[:], ident[0:2, 0:2])
            L = cp.tile([B, 2], f32, tag="L")
            nc.vector.tensor_copy(L[:], ps_sm[0:B, 0:2])
            mx = cp.tile([B, 1], f32, tag="mx")
            nc.vector.tensor_reduce(mx[:], L[:], mybir.AxisListType.X, OP.max)
            nc.vector.tensor_scalar_mul(mx[:], mx[:], -1.0)
            Ex = cp.tile([B, 2], f32, tag="Ex")
            nc.scalar.activation(Ex[:], L[:], AF.Exp, bias=mx[:])
            sm = cp.tile([B, 1], f32, tag="sm")
            nc.vector.tensor_reduce(sm[:], Ex[:], mybir.AxisListType.X, OP.add)
            nc.vector.reciprocal(sm[:], sm[:])
            nc.vector.tensor_scalar_mul(Ex[:], Ex[:], sm[:])
            nc.sync.dma_start(out[:], Ex[:])

    nc.compile()
    return nc


def kernel(**inputs) -> np.ndarray:
    K, in_maps = _prep(inputs)
    if K not in _cache:
        _cache[K] = _build(K)
    nc = _cache[K]
    res = bass_utils.run_bass_kernel_spmd(nc, in_maps,
                                          core_ids=list(range(NCORES)))
    return np.asarray(res.results[0]["out"], dtype=np.float32)


if __name__ == "__main__":
    d = np.load(os.path.join(os.path.dirname(__file__), "ref_cache.npz"))
    inputs = {k: d[k] for k in d.files if k != "expected"}
    got = kernel(**inputs)
    exp = d["expected"]
    rel = np.abs(got - exp).max() / np.abs(exp).max()
    print("Relative error:", rel)
